# revision 2
# baseline (speedup 1.0000x reference)
"""Trainium2 Bass kernel for per-(batch,head) iterative clustering attention.

Sharding: data-parallel over batch B=8 -> one batch per NeuronCore (8 cores).
All weights replicated. Inside each core: qkv projection, adaptive-avg-pool
center init, 3 soft-clustering refinement iters, cosine/sigmoid hard
assignment, aggregate + dispatch, output projection.

All matmuls run exact fp32 (fp32r is a rounded format and perturbs the
argmax-critical similarity path: 18k tokens have top-2 sim margins < 1e-3).
Softmax normalization is folded into the f operand of the center-update
matmul (same contraction index), and q is pre-normalized in stage 2 so the
sigmoid runs one batched ACT op per 5-chunk PSUM bank.
"""

import numpy as np
from contextlib import ExitStack

import concourse.bass as bass
import concourse.mybir as mybir
import concourse.tile as tile
from concourse.bass import ts
from concourse.masks import make_identity
from concourse.vector_clock import ScopedClock


def _patched_drain_and_barrier(self, tick_clock, wait_clock):
    # Walrus CTRL codegen rejects >1 sem wait on one Drain ("Too many sync
    # wait commands"); spread the tail-drain waits across extra drains.
    nc = self.nc
    drain_inst = nc.sync.drain()
    wait_clock.add_sem_waits(drain_inst.ins,
                             ScopedClock({None: tick_clock.global_clock}))
    si = drain_inst.ins.sync_info
    waits = list(si.on_wait) if si and si.on_wait else []
    MAXW = 1
    if len(waits) > MAXW:
        drain_inst.ins.sync_info = mybir.SyncInfo(
            on_wait=waits[:MAXW], on_update=list(si.on_update or []))
        rest = waits[MAXW:]
        for k in range(0, len(rest), MAXW):
            extra = nc.sync.drain()
            extra.ins.sync_info = mybir.SyncInfo(on_wait=rest[k:k + MAXW],
                                                 on_update=[])
    nc.all_engine_barrier()
    popped = nc._tile_sem_poison_stack.pop()
    assert popped is self._sem_poison
    nc.clear_and_free_semaphores(list(self.sems.allocated().values()))
    nc.all_engine_barrier()


tile.TileContext._drain_and_barrier = _patched_drain_and_barrier

_WSPLIT_ID = [0]


def split_excess_waits(nc, max_waits=1):
    """Walrus codegen rejects instructions carrying several sem waits; move
    the excess onto injected same-engine NOPs placed just before the
    offender."""
    for f in nc.m.functions:
        for bb in f.blocks:
            new = []
            for inst in bb.instructions:
                si = inst.sync_info
                waits = list(si.on_wait) if si and si.on_wait else []
                if len(waits) > max_waits:
                    keep = waits[-max_waits:]
                    rest = waits[:-max_waits]
                    for k in range(0, len(rest), max_waits):
                        _WSPLIT_ID[0] += 1
                        nop = mybir.InstNoOp(name=f"wsplit-{_WSPLIT_ID[0]}",
                                             ins=[], outs=[])
                        nop.engine = inst.engine
                        nop.sync_info = mybir.SyncInfo(
                            on_wait=rest[k:k + max_waits], on_update=[])
                        new.append(nop)
                    inst.sync_info = mybir.SyncInfo(
                        on_wait=keep, on_update=list(si.on_update or []))
                new.append(inst)
            bb.instructions[:] = new


F32 = mybir.dt.float32

HEADS = 8
HD = 64
GRID = 60
N = GRID * GRID          # 3600 tokens
C = 512
MH = MW = 10
M = MH * MW              # 100 centers
NUM_CLUSTERING = 3

TOK = 120                # token chunk (partition dim for token-major tiles)
NCH = N // TOK           # 30 chunks
KB = C // 128            # 4 contraction blocks of 128
NFC = 450                # free-dim chunk for d-major matmuls
NFN = N // NFC           # 8
G5 = 5                   # chunks per PSUM bank group
NG = NCH // G5           # 6 groups
HQ = 4                   # heads per f-stage DMA


def bclast(ap, n):
    """broadcast a [..., 1]-free AP along a new trailing free dim of size n"""
    return bass.AP(tensor=ap.tensor, offset=ap.offset,
                   ap=list(ap.ap) + [[0, n]])


def build_nc():
    nc = bass.Bass(target_bir_lowering=False)

    def ecopy(idx, out, in_):
        # alternate PSUM->SBUF copies between ACT and DVE to balance load
        if idx % 2:
            nc.scalar.copy(out=out, in_=in_)
        else:
            nc.vector.tensor_copy(out=out, in_=in_)

    x = nc.declare_dram_parameter("x", [N, C], F32, isOutput=False)
    qkv_w = nc.declare_dram_parameter("qkv_w", [C, 3 * C], F32, isOutput=False)
    qkv_b = nc.declare_dram_parameter("qkv_b", [3 * C], F32, isOutput=False)
    proj_w = nc.declare_dram_parameter("proj_w", [C, C], F32, isOutput=False)
    proj_b = nc.declare_dram_parameter("proj_b", [C], F32, isOutput=False)
    poolmat = nc.declare_dram_parameter("poolmat", [N, M], F32, isOutput=False)
    alphabeta = nc.declare_dram_parameter("alphabeta", [2], F32, isOutput=False)
    out = nc.declare_dram_parameter("out", [N, C], F32, isOutput=True)

    f_dram = nc.dram_tensor("f_stage", [N, C], F32)
    vT_dram = nc.dram_tensor("vT_stage", [128, KB, N], F32)
    qT_dram = nc.dram_tensor("qT_stage", [128, KB, N], F32)

    with tile.TileContext(nc) as tc, ExitStack() as ctx:
        singles = ctx.enter_context(tc.tile_pool(name="singles", bufs=1))

        ident = singles.tile([128, 128], F32)
        make_identity(nc, ident)

        ones_row = singles.tile([1, 128], F32)
        nc.vector.memset(ones_row, 1.0)
        ones_sb = singles.tile([TOK, 1], F32)
        nc.vector.memset(ones_sb, 1.0)

        def bcast(pool, src_ap, n_free, _bc=[0]):
            """broadcast a dram vector to all 128 partitions via K=1 matmuls
            (chunked to the 512-element fp32 moving-operand limit)"""
            t = pool.tile([128, n_free], F32, tag=f"bc{_bc[0]}")
            _bc[0] += 1
            vec = pool.tile([1, n_free], F32, tag=f"bcv{_bc[0]}")
            nc.sync.dma_start(out=vec, in_=src_ap[None, :])
            with tc.tile_pool(name=f"bcps{_bc[0]}", bufs=2, space="PSUM") as bp:
                for c in range(0, n_free, 512):
                    cw = min(512, n_free - c)
                    ps = bp.tile([128, 512], F32, tag="bps")
                    nc.tensor.matmul(ps[:, :cw], ones_row, vec[:, c:c + cw],
                                     start=True, stop=True)
                    nc.vector.tensor_copy(out=t[:, c:c + cw], in_=ps[:, :cw])
            return t

        ab_sb = bcast(singles, alphabeta[:], 2)   # [128, 2]: alpha, beta

        poolq_sb = singles.tile([M, C], F32)      # initial centers (all heads)
        poolf_sb = singles.tile([M, C], F32)      # centers_feature (all heads)

        # ============ Stage 1+2: xT, qkv, pooling, norms, staging ============
        with tc.tile_pool(name="s2", bufs=2) as s2, \
             tc.tile_pool(name="xtp", bufs=1) as xtp:

            qkvw_sb = xtp.tile([128, KB, 3 * C], F32)
            nc.sync.dma_start(out=qkvw_sb,
                              in_=qkv_w.rearrange("(j p) o -> p j o", p=128))
            qb_sb = bcast(xtp, qkv_b[:], 3 * C)       # [128, 1536]
            vbias_sb = xtp.tile([128, KB], F32)
            nc.sync.dma_start(out=vbias_sb,
                              in_=qkv_b[C:2 * C].rearrange("(j p) -> p j", p=128))

            with tc.tile_pool(name="s2ps", bufs=2, space="PSUM") as s2ps, \
                 tc.tile_pool(name="s2acc", bufs=1, space="PSUM") as s2acc, \
                 tc.tile_pool(name="s2tr", bufs=2, space="PSUM") as s2tr:
                xT = xtp.tile([128, KB, N], F32)
                poolPT = xtp.tile([TOK, NCH, M], F32)
                nc.sync.dma_start(out=poolPT,
                                  in_=poolmat.rearrange("(i p) m -> p i m", p=TOK))

                for i in range(NCH):
                    xch = s2.tile([TOK, C], F32, tag="xch")
                    nc.sync.dma_start(out=xch, in_=x[ts(i, TOK), :])
                    for j in range(KB):
                        ps = s2tr.tile([128, TOK], F32, tag="trps")
                        nc.tensor.transpose(ps, xch[:, ts(j, 128)], ident[:TOK, :TOK])
                        ecopy(j, xT[:, j, ts(i, TOK)], ps)

                poolq_ps = s2acc.tile([M, C], F32)
                poolf_ps = s2acc.tile([M, C], F32)

                for g in range(NG):                       # qT staged per 5 chunks
                    qT_sb = s2.tile([128, KB, G5 * TOK], F32, tag="qtsb")
                    for i5 in range(G5):
                        i = g * G5 + i5
                        q_ps = s2ps.tile([TOK, C], F32, tag="qf")
                        f_ps = s2ps.tile([TOK, C], F32, tag="qf")
                        for j in range(KB):
                            nc.tensor.matmul(q_ps, xT[:, j, ts(i, TOK)],
                                             qkvw_sb[:, j, 0:C],
                                             start=(j == 0), stop=(j == KB - 1))
                        for j in range(KB):
                            nc.tensor.matmul(f_ps, xT[:, j, ts(i, TOK)],
                                             qkvw_sb[:, j, 2 * C:3 * C],
                                             start=(j == 0), stop=(j == KB - 1))
                        q_i = s2.tile([TOK, C], F32, tag="qi")
                        f_i = s2.tile([TOK, C], F32, tag="fi")
                        nc.vector.tensor_add(out=q_i, in0=q_ps, in1=qb_sb[:TOK, 0:C])
                        nc.vector.tensor_add(out=f_i, in0=f_ps,
                                             in1=qb_sb[:TOK, 2 * C:3 * C])

                        nc.tensor.matmul(poolq_ps, poolPT[:, i, :], q_i,
                                         start=(i == 0), stop=(i == NCH - 1))
                        nc.tensor.matmul(poolf_ps, poolPT[:, i, :], f_i,
                                         start=(i == 0), stop=(i == NCH - 1))

                        # normalize q per token/head: qhat = q / |q|
                        sq = s2.tile([TOK, C], F32, tag="sq")
                        nc.vector.tensor_mul(out=sq, in0=q_i, in1=q_i)
                        nsq = s2.tile([TOK, HEADS], F32, tag="nsq")
                        nc.vector.reduce_sum(
                            out=nsq, in_=sq.rearrange("p (h d) -> p h d", h=HEADS),
                            axis=mybir.AxisListType.X)
                        nc.scalar.sqrt(out=nsq, in_=nsq)
                        nc.vector.reciprocal(out=nsq, in_=nsq)
                        qh = s2.tile([TOK, HEADS, HD], F32, tag="qh")
                        nc.vector.tensor_tensor(
                            qh, q_i.rearrange("p (h d) -> p h d", h=HEADS),
                            bclast(nsq, HD), mybir.AluOpType.mult)

                        qhf = qh.rearrange("p h d -> p (h d)")
                        for j in range(KB):
                            ps = s2tr.tile([128, TOK], F32, tag="trps")
                            nc.tensor.transpose(ps, qhf[:, ts(j, 128)],
                                                ident[:TOK, :TOK])
                            ecopy(j, qT_sb[:, j, ts(i5, TOK)], ps)
                        nc.sync.dma_start(out=f_dram[ts(i, TOK), :], in_=f_i)
                    nc.sync.dma_start(out=qT_dram[:, :, ts(g, G5 * TOK)], in_=qT_sb)

                nc.vector.tensor_copy(out=poolq_sb, in_=poolq_ps)
                nc.vector.tensor_copy(out=poolf_sb, in_=poolf_ps)

                # vT = Wv^T @ xT, d-major, staged to DRAM
                for fb in range(KB):
                    for t8 in range(NFN):
                        vps = s2ps.tile([128, NFC], F32, tag="vps")
                        for j in range(KB):
                            nc.tensor.matmul(
                                vps, qkvw_sb[:, j, C + fb * 128:C + (fb + 1) * 128],
                                xT[:, j, ts(t8, NFC)],
                                start=(j == 0), stop=(j == KB - 1))
                        vsb = s2.tile([128, NFC], F32, tag="vsb")
                        nc.vector.tensor_scalar_add(vsb, vps, vbias_sb[:, fb:fb + 1])
                        nc.sync.dma_start(out=vT_dram[:, fb, ts(t8, NFC)], in_=vsb)


        # ======================== per-head clustering ========================
        with tc.tile_pool(name="late", bufs=1) as late:
          outT_sb = late.tile([128, KB, N], F32)    # pre-proj out, d-major
          with tc.tile_pool(name="hd", bufs=2) as hd, \
             tc.tile_pool(name="hdbig", bufs=1) as hdbig, \
             tc.tile_pool(name="hsm", bufs=3) as hsm, \
             tc.tile_pool(name="attnP", bufs=3, space="PSUM") as attnP, \
             tc.tile_pool(name="mAcc", bufs=1, space="PSUM") as mAcc, \
             tc.tile_pool(name="trP", bufs=2, space="PSUM") as trP, \
             tc.tile_pool(name="dispP", bufs=1, space="PSUM") as dispP:

            f4 = None
            for h in range(HEADS):
                hp = 64 * (h % 2)
                hj = h // 2
                vT_h = hdbig.tile([64, N], F32, tag="vth")
                nc.sync.dma_start(out=vT_h, in_=vT_dram[hp:hp + 64, hj, :])
                qT_h = hdbig.tile([64, N], F32, tag="qth")
                nc.sync.dma_start(out=qT_h, in_=qT_dram[hp:hp + 64, hj, :])
                if h % HQ == 0:
                    f4 = hdbig.tile([TOK, NCH, HQ * HD], F32, tag="f4")
                    nc.sync.dma_start(
                        out=f4,
                        in_=f_dram.rearrange("(i p) d -> p i d", p=TOK)
                        [:, :, (h // HQ) * HQ * HD:(h // HQ + 1) * HQ * HD])
                f_h = f4[:, :, (h % HQ) * HD:(h % HQ) * HD + HD]

                centers = hd.tile([M, HD], F32, tag="ctr")
                nc.vector.tensor_copy(out=centers, in_=poolq_sb[:, ts(h, HD)])

                def transpose_centers(src):
                    ctrT = hd.tile([HD, M], F32, tag="ctrT")
                    ps = trP.tile([HD, M], F32, tag="ctps")
                    nc.tensor.transpose(ps, src, ident[:M, :M])
                    nc.vector.tensor_copy(out=ctrT, in_=ps)
                    return ctrT

                centersT = transpose_centers(centers)

                attn = hdbig.tile([TOK, NCH, M], F32, tag="attn")
                zsum = hd.tile([TOK, NCH], F32, tag="zsum")

                for it in range(NUM_CLUSTERING):
                    # iters 0/1: attn magnitude is tiny (max 2.0 / 11.6 across
                    # the workload, fp32 exp overflows at 88) -> skip max-sub
                    maxsub = True
                    for g in range(NG):
                        bank = attnP.tile([TOK, G5, M], F32, tag="aps")
                        for j5 in range(G5):
                            i = g * G5 + j5
                            nc.tensor.matmul(bank[:, j5, :], vT_h[:, ts(i, TOK)],
                                             centersT, start=True, stop=True)
                        if maxsub:
                            ngm = hsm.tile([TOK, G5], F32, tag="ngm")
                            nc.vector.tensor_reduce(out=ngm, in_=bank,
                                                    axis=mybir.AxisListType.X,
                                                    op=mybir.AluOpType.max,
                                                    negate=True)
                            sh = hsm.tile([TOK, G5, M], F32, tag="sh")
                            nc.vector.tensor_tensor(sh, bank, bclast(ngm, M),
                                                    mybir.AluOpType.add)
                            src_ap = sh
                        else:
                            src_ap = bank
                        nc.scalar.activation(
                            out=attn[:, ts(g, G5), :], in_=src_ap,
                            func=mybir.ActivationFunctionType.Exp)
                        nc.vector.reduce_sum(out=zsum[:, ts(g, G5)],
                                             in_=attn[:, ts(g, G5), :],
                                             axis=mybir.AxisListType.X)
                    rz = hd.tile([TOK, NCH], F32, tag="rz")
                    nc.vector.reciprocal(out=rz, in_=zsum)
                    cps = mAcc.tile([M, HD], F32, tag="cps")
                    for g in range(NG):
                        # fold softmax 1/Z into f along the shared contraction
                        fz5 = hsm.tile([TOK, G5, HD], F32, tag="fz")
                        nc.vector.tensor_tensor(fz5, f_h[:, ts(g, G5), :],
                                                bclast(rz[:, ts(g, G5)], HD),
                                                mybir.AluOpType.mult)
                        for j5 in range(G5):
                            i = g * G5 + j5
                            nc.tensor.matmul(cps, attn[:, i, :], fz5[:, j5, :],
                                             start=(i == 0), stop=(i == NCH - 1))
                    nc.vector.tensor_copy(out=centers, in_=cps)
                    if it < NUM_CLUSTERING - 1:
                        centersT = transpose_centers(centers)

                # normalize centers -> centers_hat^T
                cnrm = hd.tile([M, 1], F32, tag="cnrm")
                csq = hd.tile([M, HD], F32, tag="csq")
                nc.vector.tensor_mul(out=csq, in0=centers, in1=centers)
                nc.vector.reduce_sum(out=cnrm, in_=csq, axis=mybir.AxisListType.X)
                nc.scalar.sqrt(out=cnrm, in_=cnrm)
                nc.vector.tensor_scalar_max(cnrm, cnrm, 1e-12)
                nc.vector.reciprocal(out=cnrm, in_=cnrm)
                chat = hd.tile([M, HD], F32, tag="chat")
                nc.vector.tensor_scalar_mul(chat, centers, cnrm)
                chatT = transpose_centers(chat)

                # cosine similarity -> sigmoid -> argmax mask
                sim = hdbig.tile([TOK, NCH, M], F32, tag="sim")
                msim = hdbig.tile([TOK, NCH, M], F32, tag="msim")
                msk = hdbig.tile([TOK, NCH, M], F32, tag="attn")  # reuse slot
                cnt_ps = mAcc.tile([M, 1], F32, tag="cntps")
                for g in range(NG):
                    bank = attnP.tile([TOK, G5, M], F32, tag="aps")
                    for j5 in range(G5):
                        i = g * G5 + j5
                        nc.tensor.matmul(bank[:, j5, :], qT_h[:, ts(i, TOK)],
                                         chatT, start=True, stop=True)
                    nc.scalar.activation(
                        out=sim[:, ts(g, G5), :], in_=bank,
                        func=mybir.ActivationFunctionType.Sigmoid,
                        bias=ab_sb[:TOK, 1:2], scale=ab_sb[:TOK, 0:1])
                    rmx = hsm.tile([TOK, G5], F32, tag="ngm")
                    nc.vector.tensor_reduce(out=rmx, in_=sim[:, ts(g, G5), :],
                                            axis=mybir.AxisListType.X,
                                            op=mybir.AluOpType.max)
                    nc.vector.tensor_tensor(msk[:, ts(g, G5), :],
                                            sim[:, ts(g, G5), :],
                                            bclast(rmx, M),
                                            mybir.AluOpType.is_equal)
                    nc.vector.tensor_tensor(msim[:, ts(g, G5), :],
                                            sim[:, ts(g, G5), :],
                                            msk[:, ts(g, G5), :],
                                            mybir.AluOpType.mult)
                for i in range(NCH):
                    nc.tensor.matmul(cnt_ps, msk[:, i, :], ones_sb,
                                     start=(i == 0), stop=(i == NCH - 1))

                # agg
                agg_ps = mAcc.tile([M, HD], F32, tag="cps")
                for i in range(NCH):
                    nc.tensor.matmul(agg_ps, msim[:, i, :], f_h[:, i, :],
                                     start=(i == 0), stop=(i == NCH - 1))

                # out_c = (agg + poolf_h) / (count + 1)
                out_c = hd.tile([M, HD], F32, tag="outc")
                nc.vector.tensor_add(out=out_c, in0=agg_ps,
                                     in1=poolf_sb[:, ts(h, HD)])
                cnt1 = hd.tile([M, 1], F32, tag="cnt1")
                nc.vector.tensor_scalar_add(cnt1, cnt_ps, 1.0)
                nc.vector.reciprocal(out=cnt1, in_=cnt1)
                nc.vector.tensor_scalar_mul(out_c, out_c, cnt1)

                # transpose msim -> m-major for dispatch
                msimM = hdbig.tile([M, N], F32, tag="msimM")
                for i in range(NCH):
                    ps = trP.tile([M, TOK], F32, tag="ctps")
                    nc.tensor.transpose(ps, msim[:, i, :], ident[:TOK, :TOK])
                    ecopy(i, msimM[:, ts(i, TOK)], ps)

                # dispatch: out_t^T[d, n] = sum_m out_c[m, d] * msimM[m, n]
                for t8 in range(NFN):
                    dps = dispP.tile([HD, NFC], F32, tag="dps")
                    nc.tensor.matmul(dps, out_c, msimM[:, ts(t8, NFC)],
                                     start=True, stop=True)
                    ecopy(t8, outT_sb[hp:hp + 64, hj, ts(t8, NFC)], dps)

          # ============================ proj ===============================
          with tc.tile_pool(name="pr", bufs=3) as pr, \
               tc.tile_pool(name="prps", bufs=2, space="PSUM") as prps:
              projw_sb = pr.tile([128, KB, C], F32, tag="pw")
              nc.sync.dma_start(out=projw_sb,
                                in_=proj_w.rearrange("(j p) o -> p j o", p=128))
              pb_sb = bcast(pr, proj_b[:], C)
              for i in range(NCH):
                  ops = prps.tile([TOK, C], F32, tag="ops")
                  for j in range(KB):
                      nc.tensor.matmul(ops, outT_sb[:, j, ts(i, TOK)],
                                       projw_sb[:, j, :],
                                       start=(j == 0), stop=(j == KB - 1))
                  osb = pr.tile([TOK, C], F32, tag="osb")
                  nc.vector.tensor_add(out=osb, in0=ops, in1=pb_sb[:TOK, :])
                  nc.sync.dma_start(out=out[ts(i, TOK), :], in_=osb)

    split_excess_waits(nc)
    return nc


_NC_CACHE = None


def _make_poolmat():
    pm = np.zeros((N, M), dtype=np.float32)
    for bi in range(MH):
        for bj in range(MW):
            m = bi * MW + bj
            for ii in range(GRID // MH):
                for jj in range(GRID // MW):
                    n = (bi * (GRID // MH) + ii) * GRID + (bj * (GRID // MW) + jj)
                    pm[n, m] = 1.0 / ((GRID // MH) * (GRID // MW))
    return pm


def kernel(x, qkv_w, qkv_b, proj_w, proj_b, sim_alpha, sim_beta):
    from concourse.bass_utils import run_bass_kernel_spmd
    global _NC_CACHE
    if _NC_CACHE is None:
        _NC_CACHE = build_nc()
    nc = _NC_CACHE

    x = np.asarray(x, dtype=np.float32)
    B = x.shape[0]
    pm = _make_poolmat()
    ab = np.array([np.asarray(sim_alpha).reshape(-1)[0],
                   np.asarray(sim_beta).reshape(-1)[0]], dtype=np.float32)
    base = {
        "qkv_w": np.ascontiguousarray(np.asarray(qkv_w, np.float32)),
        "qkv_b": np.ascontiguousarray(np.asarray(qkv_b, np.float32)),
        "proj_w": np.ascontiguousarray(np.asarray(proj_w, np.float32)),
        "proj_b": np.ascontiguousarray(np.asarray(proj_b, np.float32)),
        "poolmat": pm,
        "alphabeta": ab,
    }
    in_maps = []
    for b in range(B):
        m = dict(base)
        m["x"] = np.ascontiguousarray(x[b].reshape(N, C))
        in_maps.append(m)
    res = run_bass_kernel_spmd(nc, in_maps, core_ids=list(range(B)))
    kernel.last_results = res
    outs = [res.results[b]["out"].reshape(GRID, GRID, C) for b in range(B)]
    return np.stack(outs, axis=0)



# revision 12
# speedup vs baseline: 1.1338x; 1.1338x over previous
"""Trainium2 Bass kernel for per-(batch,head) iterative clustering attention.

Sharding: data-parallel over batch B=8 -> one batch per NeuronCore (8 cores).
All weights replicated. Inside each core: qkv projection, adaptive-avg-pool
center init, 3 soft-clustering refinement iters, cosine/sigmoid hard
assignment, aggregate + dispatch, output projection.

All matmuls run exact fp32 (fp32r is a rounded format and perturbs the
argmax-critical similarity path: 18k tokens have top-2 sim margins < 1e-3).
Softmax normalization is folded into the f operand of the center-update
matmul (same contraction index), and q is pre-normalized in stage 2 so the
sigmoid runs one batched ACT op per 5-chunk PSUM bank.
"""

import numpy as np
from contextlib import ExitStack

import concourse.bass as bass
import concourse.mybir as mybir
import concourse.tile as tile
from concourse.bass import ts
from concourse.masks import make_identity
from concourse.vector_clock import ScopedClock


def _patched_drain_and_barrier(self, tick_clock, wait_clock):
    # Walrus CTRL codegen rejects >1 sem wait on one Drain ("Too many sync
    # wait commands"); spread the tail-drain waits across extra drains.
    nc = self.nc
    drain_inst = nc.sync.drain()
    wait_clock.add_sem_waits(drain_inst.ins,
                             ScopedClock({None: tick_clock.global_clock}))
    si = drain_inst.ins.sync_info
    waits = list(si.on_wait) if si and si.on_wait else []
    MAXW = 1
    if len(waits) > MAXW:
        drain_inst.ins.sync_info = mybir.SyncInfo(
            on_wait=waits[:MAXW], on_update=list(si.on_update or []))
        rest = waits[MAXW:]
        for k in range(0, len(rest), MAXW):
            extra = nc.sync.drain()
            extra.ins.sync_info = mybir.SyncInfo(on_wait=rest[k:k + MAXW],
                                                 on_update=[])
    nc.all_engine_barrier()
    popped = nc._tile_sem_poison_stack.pop()
    assert popped is self._sem_poison
    nc.clear_and_free_semaphores(list(self.sems.allocated().values()))
    nc.all_engine_barrier()


tile.TileContext._drain_and_barrier = _patched_drain_and_barrier

_WSPLIT_ID = [0]


def split_excess_waits(nc, max_waits=1):
    """Walrus codegen rejects instructions carrying several sem waits; move
    the excess onto injected same-engine NOPs placed just before the
    offender."""
    for f in nc.m.functions:
        for bb in f.blocks:
            new = []
            for inst in bb.instructions:
                si = inst.sync_info
                waits = list(si.on_wait) if si and si.on_wait else []
                if len(waits) > max_waits:
                    keep = waits[-max_waits:]
                    rest = waits[:-max_waits]
                    for k in range(0, len(rest), max_waits):
                        _WSPLIT_ID[0] += 1
                        nop = mybir.InstNoOp(name=f"wsplit-{_WSPLIT_ID[0]}",
                                             ins=[], outs=[])
                        nop.engine = inst.engine
                        nop.sync_info = mybir.SyncInfo(
                            on_wait=rest[k:k + max_waits], on_update=[])
                        new.append(nop)
                    inst.sync_info = mybir.SyncInfo(
                        on_wait=keep, on_update=list(si.on_update or []))
                new.append(inst)
            bb.instructions[:] = new


F32 = mybir.dt.float32
BF16 = mybir.dt.bfloat16

HEADS = 8
HD = 64
GRID = 60
N = GRID * GRID          # 3600 tokens
C = 512
MH = MW = 10
M = MH * MW              # 100 centers
NUM_CLUSTERING = 3

TOK = 120                # token chunk (partition dim for token-major tiles)
NCH = N // TOK           # 30 chunks
KB = C // 128            # 4 contraction blocks of 128
NFC = 450                # free-dim chunk for d-major matmuls
NFN = N // NFC           # 8
G5 = 5                   # chunks per PSUM bank group
NG = NCH // G5           # 6 groups
HQ = 4                   # heads per f-stage DMA


def bclast(ap, n):
    """broadcast a [..., 1]-free AP along a new trailing free dim of size n"""
    return bass.AP(tensor=ap.tensor, offset=ap.offset,
                   ap=list(ap.ap) + [[0, n]])


def build_nc():
    nc = bass.Bass(target_bir_lowering=False)

    def ecopy(idx, out, in_):
        # alternate PSUM->SBUF copies between ACT and DVE to balance load
        if idx % 2:
            nc.scalar.copy(out=out, in_=in_)
        else:
            nc.vector.tensor_copy(out=out, in_=in_)

    x = nc.declare_dram_parameter("x", [N, C], F32, isOutput=False)
    qkv_w = nc.declare_dram_parameter("qkv_w", [C, 3 * C], F32, isOutput=False)
    qkv_b = nc.declare_dram_parameter("qkv_b", [3 * C], F32, isOutput=False)
    proj_w = nc.declare_dram_parameter("proj_w", [C, C], F32, isOutput=False)
    proj_b = nc.declare_dram_parameter("proj_b", [C], F32, isOutput=False)
    poolmat = nc.declare_dram_parameter("poolmat", [N, M], F32, isOutput=False)
    alphabeta = nc.declare_dram_parameter("alphabeta", [2], F32, isOutput=False)
    out = nc.declare_dram_parameter("out", [N, C], F32, isOutput=True)

    f_dram = nc.dram_tensor("f_stage", [N, C], F32)
    vT_dram = nc.dram_tensor("vT_stage", [128, KB, N], F32)
    qT_dram = nc.dram_tensor("qT_stage", [128, KB, N], F32)

    with tile.TileContext(nc) as tc, ExitStack() as ctx:
        singles = ctx.enter_context(tc.tile_pool(name="singles", bufs=1))

        ident = singles.tile([128, 128], F32)
        make_identity(nc, ident)
        ident_bf = singles.tile([128, 128], BF16)
        make_identity(nc, ident_bf)

        ones_row = singles.tile([1, 128], F32)
        nc.vector.memset(ones_row, 1.0)
        ones_sb = singles.tile([TOK, 1], F32)
        nc.vector.memset(ones_sb, 1.0)

        def bcast(pool, src_ap, n_free, _bc=[0]):
            """broadcast a dram vector to all 128 partitions via K=1 matmuls
            (chunked to the 512-element fp32 moving-operand limit)"""
            t = pool.tile([128, n_free], F32, tag=f"bc{_bc[0]}")
            _bc[0] += 1
            vec = pool.tile([1, n_free], F32, tag=f"bcv{_bc[0]}")
            nc.sync.dma_start(out=vec, in_=src_ap[None, :])
            with tc.tile_pool(name=f"bcps{_bc[0]}", bufs=2, space="PSUM") as bp:
                for c in range(0, n_free, 512):
                    cw = min(512, n_free - c)
                    ps = bp.tile([128, 512], F32, tag="bps")
                    nc.tensor.matmul(ps[:, :cw], ones_row, vec[:, c:c + cw],
                                     start=True, stop=True)
                    nc.vector.tensor_copy(out=t[:, c:c + cw], in_=ps[:, :cw])
            return t

        ab_sb = bcast(singles, alphabeta[:], 2)   # [128, 2]: alpha, beta

        poolq_sb = singles.tile([M, C], F32)      # initial centers (all heads)
        poolf_sb = singles.tile([M, C], F32)      # centers_feature (all heads)

        # ============ Stage 1+2: xT, qkv, pooling, norms, staging ============
        with tc.tile_pool(name="s2", bufs=2) as s2, \
             tc.tile_pool(name="xtp", bufs=1) as xtp:

            qkvw_sb = xtp.tile([128, KB, 3 * C], F32)
            nc.sync.dma_start(out=qkvw_sb,
                              in_=qkv_w.rearrange("(j p) o -> p j o", p=128))
            qb_sb = bcast(xtp, qkv_b[:], 3 * C)       # [128, 1536]
            vbias_sb = xtp.tile([128, KB], F32)
            nc.sync.dma_start(out=vbias_sb,
                              in_=qkv_b[C:2 * C].rearrange("(j p) -> p j", p=128))

            with tc.tile_pool(name="s2ps", bufs=2, space="PSUM") as s2ps, \
                 tc.tile_pool(name="s2acc", bufs=1, space="PSUM") as s2acc, \
                 tc.tile_pool(name="s2tr", bufs=2, space="PSUM") as s2tr:
                xT = xtp.tile([128, KB, N], F32)
                poolPT = xtp.tile([TOK, NCH, M], F32)
                nc.sync.dma_start(out=poolPT,
                                  in_=poolmat.rearrange("(i p) m -> p i m", p=TOK))

                for i in range(NCH):
                    xch = s2.tile([TOK, C], F32, tag="xch")
                    nc.sync.dma_start(out=xch, in_=x[ts(i, TOK), :])
                    for j in range(KB):
                        ps = s2tr.tile([128, TOK], F32, tag="trps")
                        nc.tensor.transpose(ps, xch[:, ts(j, 128)], ident[:TOK, :TOK])
                        ecopy(j, xT[:, j, ts(i, TOK)], ps)

                poolq_ps = s2acc.tile([M, C], F32)
                poolf_ps = s2acc.tile([M, C], F32)

                for g in range(NG):                       # qT staged per 5 chunks
                    qT_sb = s2.tile([128, KB, G5 * TOK], F32, tag="qtsb")
                    for i5 in range(G5):
                        i = g * G5 + i5
                        q_ps = s2ps.tile([TOK, C], F32, tag="qf")
                        f_ps = s2ps.tile([TOK, C], F32, tag="qf")
                        for j in range(KB):
                            nc.tensor.matmul(q_ps, xT[:, j, ts(i, TOK)],
                                             qkvw_sb[:, j, 0:C],
                                             start=(j == 0), stop=(j == KB - 1))
                        for j in range(KB):
                            nc.tensor.matmul(f_ps, xT[:, j, ts(i, TOK)],
                                             qkvw_sb[:, j, 2 * C:3 * C],
                                             start=(j == 0), stop=(j == KB - 1))
                        q_i = s2.tile([TOK, C], F32, tag="qi")
                        f_i = s2.tile([TOK, C], F32, tag="fi")
                        nc.vector.tensor_add(out=q_i, in0=q_ps, in1=qb_sb[:TOK, 0:C])
                        nc.vector.tensor_add(out=f_i, in0=f_ps,
                                             in1=qb_sb[:TOK, 2 * C:3 * C])

                        nc.tensor.matmul(poolq_ps, poolPT[:, i, :], q_i,
                                         start=(i == 0), stop=(i == NCH - 1))
                        nc.tensor.matmul(poolf_ps, poolPT[:, i, :], f_i,
                                         start=(i == 0), stop=(i == NCH - 1))

                        # normalize q per token/head: qhat = q / |q|
                        sq = s2.tile([TOK, C], F32, tag="sq")
                        nc.vector.tensor_mul(out=sq, in0=q_i, in1=q_i)
                        nsq = s2.tile([TOK, HEADS], F32, tag="nsq")
                        nc.vector.reduce_sum(
                            out=nsq, in_=sq.rearrange("p (h d) -> p h d", h=HEADS),
                            axis=mybir.AxisListType.X)
                        nc.scalar.sqrt(out=nsq, in_=nsq)
                        nc.vector.reciprocal(out=nsq, in_=nsq)
                        qh = s2.tile([TOK, HEADS, HD], F32, tag="qh")
                        nc.vector.tensor_tensor(
                            qh, q_i.rearrange("p (h d) -> p h d", h=HEADS),
                            bclast(nsq, HD), mybir.AluOpType.mult)

                        qhf = qh.rearrange("p h d -> p (h d)")
                        for j in range(KB):
                            ps = s2tr.tile([128, TOK], F32, tag="trps")
                            nc.tensor.transpose(ps, qhf[:, ts(j, 128)],
                                                ident[:TOK, :TOK])
                            ecopy(j, qT_sb[:, j, ts(i5, TOK)], ps)
                        nc.sync.dma_start(out=f_dram[ts(i, TOK), :], in_=f_i)
                    nc.sync.dma_start(out=qT_dram[:, :, ts(g, G5 * TOK)], in_=qT_sb)

                nc.vector.tensor_copy(out=poolq_sb, in_=poolq_ps)
                nc.vector.tensor_copy(out=poolf_sb, in_=poolf_ps)

                # vT = Wv^T @ xT, d-major, staged to DRAM
                for fb in range(KB):
                    for t8 in range(NFN):
                        vps = s2ps.tile([128, NFC], F32, tag="vps")
                        for j in range(KB):
                            nc.tensor.matmul(
                                vps, qkvw_sb[:, j, C + fb * 128:C + (fb + 1) * 128],
                                xT[:, j, ts(t8, NFC)],
                                start=(j == 0), stop=(j == KB - 1))
                        vsb = s2.tile([128, NFC], F32, tag="vsb")
                        nc.vector.tensor_scalar_add(vsb, vps, vbias_sb[:, fb:fb + 1])
                        nc.sync.dma_start(out=vT_dram[:, fb, ts(t8, NFC)], in_=vsb)


        # ======================== per-head clustering ========================
        with tc.tile_pool(name="late", bufs=1) as late:
          outT_sb = late.tile([128, KB, N], BF16)   # pre-proj out, d-major
          with tc.tile_pool(name="hd", bufs=2) as hd, \
             tc.tile_pool(name="hdbig", bufs=1) as hdbig, \
             tc.tile_pool(name="hsm", bufs=3) as hsm, \
             tc.tile_pool(name="attnP", bufs=3, space="PSUM") as attnP, \
             tc.tile_pool(name="mAcc", bufs=1, space="PSUM") as mAcc, \
             tc.tile_pool(name="trP", bufs=1, space="PSUM") as trP, \
             tc.tile_pool(name="trPb", bufs=1, space="PSUM") as trPb, \
             tc.tile_pool(name="dispP", bufs=1, space="PSUM") as dispP:

            f4 = None
            f4b = None
            for h in range(HEADS):
                hp = 64 * (h % 2)
                hj = h // 2
                vT_h = hdbig.tile([64, N], F32, tag="vth")
                nc.sync.dma_start(out=vT_h, in_=vT_dram[hp:hp + 64, hj, :])
                qT_h = hdbig.tile([64, N], F32, tag="qth")
                nc.sync.dma_start(out=qT_h, in_=qT_dram[hp:hp + 64, hj, :])
                if h % HQ == 0:
                    f4 = hdbig.tile([TOK, NCH, HQ * HD], F32, tag="f4")
                    nc.sync.dma_start(
                        out=f4,
                        in_=f_dram.rearrange("(i p) d -> p i d", p=TOK)
                        [:, :, (h // HQ) * HQ * HD:(h // HQ + 1) * HQ * HD])
                    f4b = hdbig.tile([TOK, NCH, HQ * HD], BF16, tag="f4b")
                    nc.scalar.copy(out=f4b, in_=f4)
                f_h = f4[:, :, (h % HQ) * HD:(h % HQ) * HD + HD]
                fb_h = f4b[:, :, (h % HQ) * HD:(h % HQ) * HD + HD]

                centers = hd.tile([M, HD], F32, tag="ctr")
                nc.vector.tensor_copy(out=centers, in_=poolq_sb[:, ts(h, HD)])

                def transpose_centers(src):
                    ctrT = hd.tile([HD, M], F32, tag="ctrT")
                    ps = trP.tile([HD, M], F32, tag="ctps")
                    nc.tensor.transpose(ps, src, ident[:M, :M])
                    nc.vector.tensor_copy(out=ctrT, in_=ps)
                    return ctrT

                centersT = transpose_centers(centers)

                attn = hdbig.tile([TOK, NCH, M], F32, tag="attn")
                zsum = hd.tile([TOK, NCH], F32, tag="zsum")

                for it in range(NUM_CLUSTERING):
                    # iters 0/1: attn magnitude is tiny (max 2.0 / 11.6 across
                    # the workload, fp32 exp overflows at 88) -> skip max-sub
                    maxsub = True
                    for g in range(NG):
                        bank = attnP.tile([TOK, G5, M], F32, tag="aps")
                        for j5 in range(G5):
                            i = g * G5 + j5
                            nc.tensor.matmul(bank[:, j5, :], vT_h[:, ts(i, TOK)],
                                             centersT, start=True, stop=True)
                        if maxsub:
                            ngm = hsm.tile([TOK, G5], F32, tag="ngm")
                            nc.vector.tensor_reduce(out=ngm, in_=bank,
                                                    axis=mybir.AxisListType.X,
                                                    op=mybir.AluOpType.max,
                                                    negate=True)
                            sh = hsm.tile([TOK, G5, M], F32, tag="sh")
                            nc.vector.tensor_tensor(sh, bank, bclast(ngm, M),
                                                    mybir.AluOpType.add)
                            src_ap = sh
                        else:
                            src_ap = bank
                        nc.scalar.activation(
                            out=attn[:, ts(g, G5), :], in_=src_ap,
                            func=mybir.ActivationFunctionType.Exp)
                        nc.vector.reduce_sum(out=zsum[:, ts(g, G5)],
                                             in_=attn[:, ts(g, G5), :],
                                             axis=mybir.AxisListType.X)
                    rz = hd.tile([TOK, NCH], F32, tag="rz")
                    nc.vector.reciprocal(out=rz, in_=zsum)
                    cps = mAcc.tile([M, HD], F32, tag="cps")
                    for g in range(NG):
                        # fold softmax 1/Z into f along the shared contraction
                        fz5 = hsm.tile([TOK, G5, HD], F32, tag="fz")
                        nc.vector.tensor_tensor(fz5, f_h[:, ts(g, G5), :],
                                                bclast(rz[:, ts(g, G5)], HD),
                                                mybir.AluOpType.mult)
                        for j5 in range(G5):
                            i = g * G5 + j5
                            nc.tensor.matmul(cps, attn[:, i, :], fz5[:, j5, :],
                                             start=(i == 0), stop=(i == NCH - 1))
                    nc.vector.tensor_copy(out=centers, in_=cps)
                    if it < NUM_CLUSTERING - 1:
                        centersT = transpose_centers(centers)

                # normalize centers -> centers_hat^T
                cnrm = hd.tile([M, 1], F32, tag="cnrm")
                csq = hd.tile([M, HD], F32, tag="csq")
                nc.vector.tensor_mul(out=csq, in0=centers, in1=centers)
                nc.vector.reduce_sum(out=cnrm, in_=csq, axis=mybir.AxisListType.X)
                nc.scalar.sqrt(out=cnrm, in_=cnrm)
                nc.vector.tensor_scalar_max(cnrm, cnrm, 1e-12)
                nc.vector.reciprocal(out=cnrm, in_=cnrm)
                chat = hd.tile([M, HD], F32, tag="chat")
                nc.vector.tensor_scalar_mul(chat, centers, cnrm)
                chatT = transpose_centers(chat)

                # cosine similarity -> sigmoid -> argmax mask
                sim = hdbig.tile([TOK, NCH, M], F32, tag="sim")
                msim = hdbig.tile([TOK, NCH, M], BF16, tag="msim")
                msk = hdbig.tile([TOK, NCH, M], F32, tag="attn")  # reuse slot
                cnt_ps = mAcc.tile([M, 1], F32, tag="cntps")
                for g in range(NG):
                    bank = attnP.tile([TOK, G5, M], F32, tag="aps")
                    for j5 in range(G5):
                        i = g * G5 + j5
                        nc.tensor.matmul(bank[:, j5, :], qT_h[:, ts(i, TOK)],
                                         chatT, start=True, stop=True)
                    nc.scalar.activation(
                        out=sim[:, ts(g, G5), :], in_=bank,
                        func=mybir.ActivationFunctionType.Sigmoid,
                        bias=ab_sb[:TOK, 1:2], scale=ab_sb[:TOK, 0:1])
                    rmx = hsm.tile([TOK, G5], F32, tag="ngm")
                    nc.vector.tensor_reduce(out=rmx, in_=sim[:, ts(g, G5), :],
                                            axis=mybir.AxisListType.X,
                                            op=mybir.AluOpType.max)
                    nc.vector.tensor_tensor(msk[:, ts(g, G5), :],
                                            sim[:, ts(g, G5), :],
                                            bclast(rmx, M),
                                            mybir.AluOpType.is_equal)
                    nc.vector.tensor_tensor(msim[:, ts(g, G5), :],
                                            sim[:, ts(g, G5), :],
                                            msk[:, ts(g, G5), :],
                                            mybir.AluOpType.mult)
                for i in range(NCH):
                    nc.tensor.matmul(cnt_ps, msk[:, i, :], ones_sb,
                                     start=(i == 0), stop=(i == NCH - 1))

                # agg (bf16: post-argmax values tolerate it)
                agg_ps = mAcc.tile([M, HD], F32, tag="cps")
                for i in range(NCH):
                    nc.tensor.matmul(agg_ps, msim[:, i, :], fb_h[:, i, :],
                                     start=(i == 0), stop=(i == NCH - 1))

                # out_c = (agg + poolf_h) / (count + 1)
                out_c = hd.tile([M, HD], F32, tag="outc")
                nc.vector.tensor_add(out=out_c, in0=agg_ps,
                                     in1=poolf_sb[:, ts(h, HD)])
                cnt1 = hd.tile([M, 1], F32, tag="cnt1")
                nc.vector.tensor_scalar_add(cnt1, cnt_ps, 1.0)
                nc.vector.reciprocal(out=cnt1, in_=cnt1)
                nc.vector.tensor_scalar_mul(out_c, out_c, cnt1)

                # transpose msim -> m-major for dispatch (bf16)
                msimM = hdbig.tile([M, N], BF16, tag="msimM")
                for i in range(NCH):
                    ps = trPb.tile([M, TOK], BF16, tag="ctpsb")
                    nc.tensor.transpose(ps, msim[:, i, :], ident_bf[:TOK, :TOK])
                    ecopy(i, msimM[:, ts(i, TOK)], ps)

                out_cb = hd.tile([M, HD], BF16, tag="outcb")
                nc.vector.tensor_copy(out=out_cb, in_=out_c)

                # dispatch: out_t^T[d, n] = sum_m out_c[m, d] * msimM[m, n]
                for t8 in range(NFN):
                    dps = dispP.tile([HD, NFC], F32, tag="dps")
                    nc.tensor.matmul(dps, out_cb, msimM[:, ts(t8, NFC)],
                                     start=True, stop=True)
                    ecopy(t8, outT_sb[hp:hp + 64, hj, ts(t8, NFC)], dps)

          # ============================ proj ===============================
          with tc.tile_pool(name="pr", bufs=3) as pr, \
               tc.tile_pool(name="prps", bufs=2, space="PSUM") as prps:
              projw_f = pr.tile([128, KB, C], F32, tag="pwf")
              nc.sync.dma_start(out=projw_f,
                                in_=proj_w.rearrange("(j p) o -> p j o", p=128))
              projw_sb = pr.tile([128, KB, C], BF16, tag="pw")
              nc.scalar.copy(out=projw_sb, in_=projw_f)
              pb_sb = bcast(pr, proj_b[:], C)
              for i in range(NCH):
                  ops = prps.tile([TOK, C], F32, tag="ops")
                  for j in range(KB):
                      nc.tensor.matmul(ops, outT_sb[:, j, ts(i, TOK)],
                                       projw_sb[:, j, :],
                                       start=(j == 0), stop=(j == KB - 1))
                  osb = pr.tile([TOK, C], F32, tag="osb")
                  nc.vector.tensor_add(out=osb, in0=ops, in1=pb_sb[:TOK, :])
                  nc.sync.dma_start(out=out[ts(i, TOK), :], in_=osb)

    split_excess_waits(nc)
    return nc


_NC_CACHE = None


def _make_poolmat():
    pm = np.zeros((N, M), dtype=np.float32)
    for bi in range(MH):
        for bj in range(MW):
            m = bi * MW + bj
            for ii in range(GRID // MH):
                for jj in range(GRID // MW):
                    n = (bi * (GRID // MH) + ii) * GRID + (bj * (GRID // MW) + jj)
                    pm[n, m] = 1.0 / ((GRID // MH) * (GRID // MW))
    return pm


def kernel(x, qkv_w, qkv_b, proj_w, proj_b, sim_alpha, sim_beta):
    from concourse.bass_utils import run_bass_kernel_spmd
    global _NC_CACHE
    if _NC_CACHE is None:
        _NC_CACHE = build_nc()
    nc = _NC_CACHE

    x = np.asarray(x, dtype=np.float32)
    B = x.shape[0]
    pm = _make_poolmat()
    ab = np.array([np.asarray(sim_alpha).reshape(-1)[0],
                   np.asarray(sim_beta).reshape(-1)[0]], dtype=np.float32)
    base = {
        "qkv_w": np.ascontiguousarray(np.asarray(qkv_w, np.float32)),
        "qkv_b": np.ascontiguousarray(np.asarray(qkv_b, np.float32)),
        "proj_w": np.ascontiguousarray(np.asarray(proj_w, np.float32)),
        "proj_b": np.ascontiguousarray(np.asarray(proj_b, np.float32)),
        "poolmat": pm,
        "alphabeta": ab,
    }
    in_maps = []
    for b in range(B):
        m = dict(base)
        m["x"] = np.ascontiguousarray(x[b].reshape(N, C))
        in_maps.append(m)
    res = run_bass_kernel_spmd(nc, in_maps, core_ids=list(range(B)))
    kernel.last_results = res
    outs = [res.results[b]["out"].reshape(GRID, GRID, C) for b in range(B)]
    return np.stack(outs, axis=0)

